# revision 53
# baseline (speedup 1.0000x reference)
"""Trainium2 Bass kernel for nn_MixGNN (TransformerConv + 3x SAGEConv + BN + gated residual).

Strategy (8 NeuronCores, dst-node sharding):
  - Pad N 10000 -> 10240; core r owns 1280 dst nodes = 10 tiles of 128. Each
    core's tile slots are sorted by edge count (descending) so the static SPMD
    per-slot chunk counts (max over cores) stay tight; all DRAM node tables
    are stored slot-ordered and gather indices are remapped to positions.
  - Host preprocessing (graph structure + parameter algebra only): sort edges
    by dst, bucket per dst-tile, pad to 128-edge chunks, wrapped int16 gather
    indices, per-chunk local-dst + invdeg-of-dst columns, packed bf16 weights.
    Attention is folded: M = Wq @ Wk.T / sqrt(d), so logits[e] =
    x[dst_e] @ M @ x[src_e]^T; the bk term is per-dst and cancels in softmax;
    bq is zero in this problem. BN gamma (eval mode) is folded into Wl/Wr
    columns; all biases enter PSUM via rank-1 ones-row matmuls.
  - Transformer pass: per tile, dma_gather the x-table twice (transposed for
    scores, rows for values); scores psc = xgT.T @ (M^T X_tile^T) on PE in
    4-chunk groups; exp on Act per group (no max-subtraction: logits O(1));
    w_b = (dst_e==n) * exp on DVE; value-agg + exp-sum denominator via
    indicator matmuls into PSUM; normalize, then post-multiply by Wv
    (linearity) together with the x @ Ws skip path.
  - SAGE pass: one gather per layer from the AllGathered bf16 h table;
    indicator build folds 1/deg (per-edge scalar); aggregation is TRANSPOSED
    (paggT[d,n] += vg[e,d]^T @ ind[e,n], separate PSUM banks per d-chunk) so
    the mean feeds Wl matmuls directly without PE transposes; h @ Wr uses a
    transposed h copy maintained per tile; gated residual + ReLU epilogue.
  - Gathers are split into ~4 even pieces per tile to pipeline SWDGE
    descriptor-gen (Pool) against DMA transfer.
  - Halo exchange: 3 AllGathers (h0, h1, h2) of 0.65MB/core bf16 shards.
Output: fp32 [10000, 256] (bf16 device output, upcast + slot-unpermuted on host).
"""
import os
import sys
import time

import numpy as np

for _p in ("/opt/trn_rl_repo",):
    if _p not in sys.path:
        sys.path.insert(0, _p)

import ml_dtypes  # noqa: E402
import concourse.bacc as bacc  # noqa: E402
import concourse.mybir as mybir  # noqa: E402
import concourse.tile as tile  # noqa: E402
from concourse.bass_utils import run_bass_kernel_spmd  # noqa: E402

P = 128
D = 256
DJ = D // P           # 2 d-chunks of 128
NC = 8                # cores
L = 3                 # SAGE layers
BN_EPS = 1e-5
N_AG = 3              # AllGathers on the critical path (h0, h1, h2)

F32 = mybir.dt.float32
BF16 = mybir.dt.bfloat16
I16 = mybir.dt.int16
V_DT = BF16           # gathered-table + indicator dtype
H_DT = BF16

_nc_cache = {}


def _wrap_idx(a):
    """[S*128] int array -> [128, S*8] int16 wrapped gather-index layout."""
    w16 = a.reshape(-1, 16).T.astype(np.int16)   # [16, S*8]
    return np.tile(w16, (8, 1))                  # replicate to 8 Q7 stripes


def build_nc(n_pad, sh, nt, S_list, scale, oma):
    stages = int(os.environ.get("KSTAGES", "5"))
    nocc = os.environ.get("KNOCC") == "1"
    ksm = int(os.environ.get("KSM", "12"))
    kgp = int(os.environ.get("KGP", "2"))
    kpsc = int(os.environ.get("KPSC", "3"))
    kptr = int(os.environ.get("KPTR", "1"))
    kpagg = int(os.environ.get("KPAGG", "2"))
    kpmm = int(os.environ.get("KPMM", "2"))
    khalf = int(os.environ.get("KHALF", "4"))  # gather splits per tile
    kabl = os.environ.get("KABL", "")
    S_list = tuple(int(s) for s in S_list)
    key = (n_pad, sh, nt, S_list, round(scale, 9), round(oma, 9), stages,
           nocc, ksm, kgp, kpsc, kptr, kpagg, kpmm, khalf, kabl,
           os.environ.get("KHALFT"),
           os.environ.get("KKGT"), os.environ.get("KVG"), os.environ.get("KPAIR"), os.environ.get("KGRAD"), os.environ.get("KTAIL"), os.environ.get("KTAIL2"))
    if key in _nc_cache:
        return _nc_cache[key]

    SC = sum(S_list)                 # total chunks across local tiles
    offs = [0]
    for s in S_list:
        offs.append(offs[-1] + s)
    ndev = 1 if nocc else NC
    nc = bacc.Bacc("TRN2", target_bir_lowering=False, debug=False, num_devices=ndev)

    NW = 9  # packed weights: M, Wv, Ws, Wl0, Wr0, Wl1, Wr1, Wl2, Wr2
    NV = 7  # packed vecs: bv+bs, Gx0, Bx0, Gx1, Bx1, Gx2, Bx2

    xt_in = nc.dram_tensor("xt_in", [P, DJ * sh], BF16, kind="ExternalInput")
    wpack_in = nc.dram_tensor("wpack_in", [P, NW * DJ * D], BF16, kind="ExternalInput")
    vpack_in = nc.dram_tensor("vpack_in", [1, NV * D], BF16, kind="ExternalInput")
    idx_in = nc.dram_tensor("idx_in", [P, SC * 8], I16, kind="ExternalInput")
    dst_in = nc.dram_tensor("dst_in", [P, 2 * SC], F32, kind="ExternalInput")
    xtab_in = nc.dram_tensor("xtab_in", [n_pad, D], BF16, kind="ExternalInput")
    out_dram = nc.dram_tensor("out", [sh, D], BF16, kind="ExternalOutput")

    WM, WV, WS = 0, 1, 2
    WL = [3, 5, 7]
    WR = [4, 6, 8]
    VBS = 0

    with tile.TileContext(nc) as tc:
        with (
            tc.tile_pool(name="cst", bufs=1) as cst,
            tc.tile_pool(name="sb", bufs=1) as sb,
            tc.tile_pool(name="g", bufs=kgp) as gp,
            tc.tile_pool(name="sm", bufs=ksm) as smp,
            tc.tile_pool(name="ps", bufs=2, space="PSUM") as ps,
            tc.tile_pool(name="dr", bufs=1, space="DRAM") as dr,
        ):
            # ---------------- constants / inputs to SBUF ----------------
            idx_sb = cst.tile([P, SC * 8], I16)
            _ic = S_list[0] * 8  # first tile's indices land first
            nc.sync.dma_start(out=idx_sb[:, :_ic], in_=idx_in[:, :_ic])
            nc.sync.dma_start(out=idx_sb[:, _ic:], in_=idx_in[:, _ic:])
            dstc = cst.tile([P, 2 * SC], F32)
            nc.sync.dma_start(out=dstc[:], in_=dst_in[:])
            wp = cst.tile([P, NW * DJ * D], BF16)
            nc.sync.dma_start(out=wp[:], in_=wpack_in[:])
            vp = cst.tile([1, NV * D], BF16)
            nc.sync.dma_start(out=vp[:], in_=vpack_in[:])
            xt = cst.tile([P, DJ * sh], BF16)
            for _xi in range(4):
                _c0 = _xi * (DJ * sh // 4)
                _c1 = (_xi + 1) * (DJ * sh // 4)
                nc.sync.dma_start(out=xt[:, _c0:_c1], in_=xt_in[:, _c0:_c1])

            iota_i = cst.tile([P, P], mybir.dt.int32)
            nc.gpsimd.iota(iota_i[:], pattern=[[1, P]], base=0, channel_multiplier=0)
            ones_v = cst.tile([P, 1], V_DT)
            nc.vector.memset(ones_v[:], 1.0)
            ones_row = cst.tile([1, P], BF16)
            nc.vector.memset(ones_row[:], 1.0)
            # identity for PE transposes: (iota_row == partition_idx)
            iota_part = cst.tile([P, 1], mybir.dt.int32)
            nc.gpsimd.iota(iota_part[:], pattern=[[1, 1]], base=0, channel_multiplier=1)
            iota_part_f = cst.tile([P, 1], F32)
            nc.vector.tensor_copy(out=iota_part_f[:], in_=iota_part[:])
            iota_f = cst.tile([P, P], F32)
            nc.vector.tensor_copy(out=iota_f[:], in_=iota_i[:])
            ident = cst.tile([P, P], F32)
            nc.vector.tensor_scalar(
                out=ident[:], in0=iota_f[:], scalar1=iota_part_f[:, :1], scalar2=None,
                op0=mybir.AluOpType.is_equal,
            )
            ident_b = cst.tile([P, P], BF16)
            nc.vector.tensor_copy(out=ident_b[:], in_=ident[:])
            iota_b = cst.tile([P, P], BF16)
            nc.vector.tensor_copy(out=iota_b[:], in_=iota_f[:])

            def wslice(w, j):
                return wp[:, (w * DJ + j) * D:(w * DJ + j + 1) * D]

            def vslice(k):
                return vp[:, k * D:(k + 1) * D]  # [1, D] single-partition row

            def xtile(j, t):
                return xt[:, j * sh + t * P: j * sh + (t + 1) * P]

            # ---------------- DRAM tables ----------------
            hag_in = [dr.tile([sh, D], H_DT, name=f"hag_in_{i}") for i in range(L)]
            h_full = [dr.tile([n_pad, D], H_DT, name=f"h_full_{i}",
                              addr_space=("Local" if nocc else "Shared"))
                      for i in range(L)]

            def allgather(in_t, out_t):
                if nocc:
                    pass  # per-tile h_full writes above stand in for the AG
                else:
                    nc.gpsimd.collective_compute(
                        "AllGather", mybir.AluOpType.bypass,
                        replica_groups=[list(range(NC))],
                        ins=[in_t[:]], outs=[out_t[:]],
                    )

            # ---------------- stage 0: aT = M^T X_tile^T per tile ----------------
            # aT[j][d, n] (j-th 128-row chunk of d) so that
            # psc[e, n] = sum_d xgT[d, e] * aT[d, n] = (x[src_e] @ M^T) . x[n]
            #           = x[n] @ M @ x[src_e]^T  (logit of edge e -> dst n)
            aT = [sb.tile([P, sh], BF16, name=f"aT_{j}") for j in range(DJ)]
            n0 = 0
            while n0 < sh:
                nn = min(512, sh - n0)
                for j in range(DJ):
                    pq = ps.tile([P, 512], F32, name="pq", tag="pmm", bufs=kpmm)
                    for ki in range(DJ):
                        nc.tensor.matmul(
                            pq[:, :nn],
                            lhsT=wslice(WM, ki)[:, j * P:(j + 1) * P],
                            rhs=xt[:, ki * sh + n0: ki * sh + n0 + nn],
                            start=(ki == 0), stop=(ki == DJ - 1),
                        )
                    nc.scalar.copy(out=aT[j][:, n0:n0 + nn], in_=pq[:, :nn])
                n0 += nn

            # shard-resident activations
            h_cur = sb.tile([P, nt * D], H_DT)
            h_nxt = sb.tile([P, nt * D], H_DT)
            hT_cur = sb.tile([P, DJ * sh], BF16)
            hT_nxt = sb.tile([P, DJ * sh], BF16)

            def agg_pass(layer, h_prev, hT_prev, h_out, hT_out):
                """layer -1: transformer (h_prev/hT_prev unused); 0..L-1: SAGE."""
                li = layer + 1  # h table index this pass WRITES (0 for transformer)
                kh = khalf if layer >= 0 else int(os.environ.get("KHALFT", "4"))
                for t in range(nt):
                    St = S_list[t]
                    ETt = St * P
                    o8 = offs[t] * 8
                    splits = []  # (c0, c1) chunk ranges per gather piece
                    c0 = 0
                    hi = St
                    tail_ws = []
                    _tw = os.environ.get("KTAIL", "4,2,2")
                    _tws = tuple(int(w) for w in _tw.split(",") if w)
                    if t == nt - 2 and os.environ.get("KTAIL2", "") and St > 6:
                        _t2 = tuple(int(w) for w in
                                    os.environ["KTAIL2"].split(",") if w)
                        tail_ws = list(_t2)
                        hi -= sum(_t2)
                    if t == nt - 1 and St > sum(_tws) + 2 and _tws:
                        tail_ws = list(_tws)
                        hi -= sum(_tws)
                    if t == 0:
                        _grad = tuple(int(w) for w in
                                      os.environ.get("KGRAD", "").split(",") if w)
                        for w in _grad:  # small leading pieces: lower latency
                            splits.append((c0, min(St, c0 + w)))
                            c0 += w
                            if c0 >= St:
                                break
                    base = max(1, (hi - c0 + kh - 1) // kh)
                    while c0 < hi:
                        ce = min(hi, c0 + base)
                        splits.append((c0, ce))
                        c0 = ce
                    for w in tail_ws:
                        splits.append((c0, c0 + w))
                        c0 += w
                    if layer < 0:
                        vg = gp.tile([P, St, D], V_DT, name="vg", tag="vg",
                                     bufs=int(os.environ.get("KVG", "3")))
                    else:
                        vg = gp.tile([P, St, D], H_DT, name="hg", tag="vg",
                                     bufs=int(os.environ.get("KVG", "3")))
                    kgt_pieces = []
                    if layer < 0:
                        ksplits = [s for s in splits]
                        nkg = 2 * kh + 4
                        for (ck, ce) in ksplits:
                            nn_k = (ce - ck) * P
                            nn_k = (ce - ck) * P
                            kgp_t = gp.tile([P, DJ, nn_k], BF16, name="kgt",
                                            tag="kgt", bufs=nkg)
                            nc.gpsimd.dma_gather(
                                out_ap=kgp_t[:],
                                in_ap=xtab_in[:],
                                idxs_ap=idx_sb[:, o8 + ck * 8: o8 + ce * 8],
                                num_idxs=nn_k, num_idxs_reg=nn_k, elem_size=D,
                                transpose=True, single_packet=False)
                            kgt_pieces.append((ck, ce, kgp_t))
                    src_tab = xtab_in if layer < 0 else h_full[layer]
                    for (ca, cb) in splits:
                        nn_i = (cb - ca) * P
                        idx_t = idx_sb[:, o8 + ca * 8: o8 + cb * 8]
                        nc.gpsimd.dma_gather(
                            out_ap=vg[:, ca:cb, :], in_ap=src_tab[:], idxs_ap=idx_t,
                            num_idxs=nn_i, num_idxs_reg=nn_i, elem_size=D,
                            single_packet=False)

                    if layer < 0:
                        pagg = ps.tile([P, D + 1], F32, name="pagg", tag="pagg",
                                       bufs=kpagg)
                        pz = ps.tile([P, D], F32, name="pz", tag="pmm", bufs=kpmm)
                        nc.tensor.matmul(pz[:], lhsT=ones_row[:], rhs=vslice(VBS),
                                         start=True, stop=False)
                        for j in range(DJ):
                            nc.tensor.matmul(pz[:], lhsT=xtile(j, t),
                                             rhs=wslice(WS, j),
                                             start=False, stop=False)
                    else:
                        # transposed agg: separate PSUM tiles per d-chunk
                        # (start=True zeroes a whole bank; slices can't share)
                        paggT = [ps.tile([P, P], F32, name=f"paggT{j}", tag="psc",
                                         bufs=kpsc) for j in range(DJ)]
                        pz = ps.tile([P, D], F32, name="pz", tag="pmm", bufs=kpmm)
                        nc.tensor.matmul(pz[:], lhsT=ones_row[:],
                                         rhs=vslice(2 + 2 * layer),
                                         start=True, stop=False)
                        for j in range(DJ):
                            nc.tensor.matmul(
                                pz[:],
                                lhsT=hT_prev[:, j * sh + t * P: j * sh + (t + 1) * P],
                                rhs=wslice(WR[layer], j),
                                start=False, stop=False)
                    if layer < 0:
                        # chunk pairs: one [P,2P] exp per two chunks (halves
                        # the Act per-instruction init overhead)
                        kpair = int(os.environ.get("KPAIR", "4"))
                        cp = 0
                        while cp < St:
                            npair = min(kpair, St - cp)
                            psc = ps.tile([P, npair * P], F32, name="psc",
                                          tag="psc", bufs=kpsc)
                            for ci in range(npair):
                                c = cp + ci
                                kge = next(p for p in kgt_pieces
                                           if p[0] <= c < p[1])
                                cof = c - kge[0]
                                for j in range(DJ):
                                    nc.tensor.matmul(
                                        psc[:, ci * P:(ci + 1) * P],
                                        lhsT=kge[2][:, j, cof * P:(cof + 1) * P],
                                        rhs=aT[j][:, t * P:(t + 1) * P],
                                        start=(j == 0), stop=(j == DJ - 1))
                            exps = smp.tile([P, npair * P], BF16, name="exps")
                            nc.scalar.activation(exps[:], psc[:],
                                                 mybir.ActivationFunctionType.Exp)
                            for ci in range(npair):
                                c = cp + ci
                                dcol = dstc[:, offs[t] + c: offs[t] + c + 1]
                                w_b = smp.tile([P, P], V_DT, name="w_b", tag="w_b")
                                nc.vector.scalar_tensor_tensor(
                                    out=w_b[:], in0=iota_b[:], scalar=dcol,
                                    in1=exps[:, ci * P:(ci + 1) * P],
                                    op0=mybir.AluOpType.is_equal,
                                    op1=mybir.AluOpType.mult)
                                nc.tensor.matmul(pagg[:, :D], lhsT=w_b[:],
                                                 rhs=vg[:, c, :],
                                                 start=(c == 0), stop=(c == St - 1))
                                nc.tensor.matmul(pagg[:, D:D + 1], lhsT=w_b[:],
                                                 rhs=ones_v[:],
                                                 start=False, stop=(c == St - 1))
                            cp += npair
                    else:
                        for c in range(St):
                            dcol = dstc[:, offs[t] + c: offs[t] + c + 1]
                            ivcol = dstc[:, SC + offs[t] + c: SC + offs[t] + c + 1]
                            ind_b = smp.tile([P, P], H_DT, name="ind_b", tag="w_b")
                            nc.vector.tensor_scalar(
                                out=ind_b[:], in0=iota_b[:], scalar1=dcol,
                                scalar2=ivcol, op0=mybir.AluOpType.is_equal,
                                op1=mybir.AluOpType.mult)
                            for j in range(DJ):
                                nc.tensor.matmul(
                                    paggT[j][:],
                                    lhsT=vg[:, c, j * P:(j + 1) * P],
                                    rhs=ind_b[:],
                                    start=(c == 0), stop=(c == St - 1))

                    # ---- tile epilogue -> h_out tile [node, d] ----
                    if layer < 0:
                        smax = smp.tile([P, 1], F32, name="smax")
                        nc.vector.tensor_scalar(
                            out=smax[:], in0=pagg[:, D:D + 1], scalar1=1e-30,
                            scalar2=None, op0=mybir.AluOpType.max)
                        rs = smp.tile([P, 1], F32, name="rs")
                        nc.vector.reciprocal(rs[:], smax[:])
                        # mean_x = (sum_e attn * x[src]) / denom, then
                        # h = relu(mean_x @ Wv + x @ Ws + (bv + bs))
                        mean_x = smp.tile([P, D], BF16, name="mean_x", tag="t1")
                        nc.scalar.activation(mean_x[:], pagg[:, :D],
                                             mybir.ActivationFunctionType.Copy,
                                             scale=rs[:, :1])
                        for j in range(DJ):
                            ptr = ps.tile([P, P], BF16, name="ptr", tag="ptr", bufs=kptr)
                            nc.tensor.transpose(out=ptr[:],
                                                in_=mean_x[:, j * P:(j + 1) * P],
                                                identity=ident_b[:])
                            mT = smp.tile([P, P], BF16, name="mT", tag="mT")
                            nc.scalar.copy(out=mT[:], in_=ptr[:])
                            nc.tensor.matmul(pz[:], lhsT=mT[:],
                                             rhs=wslice(WV, j),
                                             start=False, stop=(j == DJ - 1))
                        nc.scalar.activation(h_out[:, t * D:(t + 1) * D], pz[:],
                                             mybir.ActivationFunctionType.Relu)
                        hfin = None
                    else:
                        for j in range(DJ):
                            mT = smp.tile([P, P], BF16, name="mT", tag="mT")
                            nc.scalar.copy(out=mT[:], in_=paggT[j][:])
                            nc.tensor.matmul(pz[:], lhsT=mT[:],
                                             rhs=wslice(WL[layer], j),
                                             start=False, stop=(j == DJ - 1))
                        t3 = smp.tile([P, D], F32, name="t3s", tag="t4")
                        nc.vector.scalar_tensor_tensor(
                            out=t3[:], in0=h_prev[:, t * D:(t + 1) * D], scalar=oma,
                            in1=pz[:], op0=mybir.AluOpType.mult,
                            op1=mybir.AluOpType.add)
                        if layer < L - 1:
                            nc.scalar.activation(h_out[:, t * D:(t + 1) * D], t3[:],
                                                 mybir.ActivationFunctionType.Relu)
                        else:
                            hfin = smp.tile([P, D], BF16, name="hfin", tag="t1")
                            nc.scalar.activation(hfin[:], t3[:],
                                                 mybir.ActivationFunctionType.Relu)

                    if layer < L - 1:
                        if nocc:
                            # sim stand-in: the collective's local table write,
                            # fed straight from the shard epilogue
                            nc.sync.dma_start(out=h_full[li][t * P:(t + 1) * P, :],
                                              in_=h_out[:, t * D:(t + 1) * D])
                        else:
                            nc.sync.dma_start(out=hag_in[li][t * P:(t + 1) * P, :],
                                              in_=h_out[:, t * D:(t + 1) * D])
                        for j in range(DJ):
                            ptr2 = ps.tile([P, P], H_DT, name="ptr2", tag="ptr", bufs=kptr)
                            nc.tensor.transpose(
                                out=ptr2[:],
                                in_=h_out[:, t * D + j * P: t * D + (j + 1) * P],
                                identity=ident_b[:])
                            nc.scalar.copy(
                                out=hT_out[:, j * sh + t * P: j * sh + (t + 1) * P],
                                in_=ptr2[:])
                    else:
                        nc.sync.dma_start(out=out_dram[t * P:(t + 1) * P, :],
                                          in_=hfin[:])

                if layer < L - 1:
                    allgather(hag_in[li], h_full[li])

            if stages <= 1:
                # dump a slice so the program has an output
                tmpo = smp.tile([P, D], F32, name="tmpo")
                for t in range(nt):
                    nc.vector.tensor_copy(out=tmpo[:], in_=xt[:, :D])
                    nc.sync.dma_start(out=out_dram[t * P:(t + 1) * P, :], in_=tmpo[:])
            else:
                agg_pass(-1, None, None, h_cur, hT_cur)
                bufs = [(h_cur, hT_cur), (h_nxt, hT_nxt)]
                for i in range(min(L, stages - 2)):
                    h_prev, hT_prev = bufs[i % 2]
                    h_out, hT_out = bufs[(i + 1) % 2]
                    agg_pass(i, h_prev, hT_prev, h_out, hT_out)
                if stages - 2 < L:
                    hsrc, _ = bufs[max(0, stages - 2) % 2]
                    for t in range(nt):
                        nc.sync.dma_start(out=out_dram[t * P:(t + 1) * P, :],
                                          in_=hsrc[:, t * D:(t + 1) * D])

    nc.compile()
    _nc_cache[key] = nc
    return nc


def _host_prep(x, src, dst, Wq, bq, Wk, bk, Wv, bv, Ws, bs, Wl, bl, Wr,
               gamma, beta, alpha_res):
    n, d = x.shape
    n_pad = ((n + NC * P - 1) // (NC * P)) * (NC * P)
    sh = n_pad // NC
    nt = sh // P
    n_tiles = n_pad // P

    order = np.argsort(dst, kind="stable")
    src_s, dst_s = src[order], dst[order]
    tile_of = dst_s // P
    counts = np.bincount(tile_of, minlength=n_tiles)
    starts = np.concatenate([[0], np.cumsum(counts)])

    # Per-core slot assignment: sort each core's local tiles by edge count
    # (descending) so slot k holds every core's k-th busiest tile. The static
    # SPMD chunk count per slot is then the max over cores, which is tight.
    perms = []   # perms[r][k] = local tile index of core r in slot k
    s_sorted = np.empty((NC, nt), np.int64)
    for r in range(NC):
        c_r = counts[r * nt:(r + 1) * nt]
        p_r = np.argsort(-c_r, kind="stable")
        perms.append(p_r)
        s_sorted[r] = (c_r[p_r] + P - 1) // P
    S_list = np.maximum(s_sorted.max(axis=0), 1).astype(np.int64)
    SC = int(S_list.sum())
    offs = np.concatenate([[0], np.cumsum(S_list)]).astype(np.int64)

    # All DRAM node tables (xtab, h_full via hag_in writes) are slot-ordered:
    # position (r*nt + k)*P + p holds node (r*nt + perms[r][k])*P + p. Gather
    # indices address table positions, so remap node ids -> positions.
    invperms = [np.argsort(p) for p in perms]
    pos_of_tile = np.empty(n_tiles, np.int64)
    for r in range(NC):
        pos_of_tile[r * nt:(r + 1) * nt] = r * nt + invperms[r]
    ar = np.arange(n_pad)
    pos_of_node = pos_of_tile[ar // P] * P + (ar % P)
    src_pos = pos_of_node[src_s]

    deg = np.bincount(dst, minlength=n_pad).astype(np.float32)
    invdeg_full = 1.0 / np.maximum(deg, 1.0)

    al = 1.0 / (1.0 + np.exp(-alpha_res))
    oma = float(1.0 - al)
    bn_scale = 1.0 / np.sqrt(1.0 + BN_EPS)
    scale = 1.0 / np.sqrt(float(d))

    x_pad = np.zeros((n_pad, D), np.float32)
    x_pad[:n] = x
    xT = x_pad.T.copy()
    xtab = np.zeros((n_pad, D), ml_dtypes.bfloat16)
    xtab[pos_of_node] = x_pad.astype(ml_dtypes.bfloat16)

    # fold attention: logits = scale * (x[dst] @ Wq + bq) . (x[src] @ Wk + bk)
    #   = x[dst] @ M @ x[src]^T  (+ per-dst const, cancels in softmax; bq = 0)
    M = (Wq @ Wk.T) * scale
    Gx = [al * bn_scale * gamma[i] for i in range(L)]
    Wlg = [Wl[i] * Gx[i][None, :] for i in range(L)]
    Wrg = [Wr[i] * Gx[i][None, :] for i in range(L)]
    weights = [M, Wv, Ws, Wlg[0], Wrg[0], Wlg[1], Wrg[1], Wlg[2], Wrg[2]]
    NW = len(weights)
    wpack = np.empty((P, NW * DJ * D), np.float32)
    for w, W in enumerate(weights):
        for j in range(DJ):
            wpack[:, (w * DJ + j) * D:(w * DJ + j + 1) * D] = W[j * P:(j + 1) * P, :]
    wpack = wpack.astype(ml_dtypes.bfloat16)

    Bx = [al * (bl[i] * bn_scale * gamma[i] + beta[i]) for i in range(L)]
    vecs = [bv + bs, Bx[0], Bx[0], Bx[1], Bx[1], Bx[2], Bx[2]]
    vpack = np.concatenate(vecs)[None, :].astype(ml_dtypes.bfloat16)

    in_maps = []
    for r in range(NC):
        idx_arr = np.zeros((P, SC * 8), np.int16)
        dst_arr = np.full((P, 2 * SC), 128.0, np.float32)
        dst_arr[:, SC:] = 0.0
        for k in range(nt):
            tloc = int(perms[r][k])
            St = int(S_list[k])
            ETt = St * P
            g = r * nt + tloc
            e0, e1 = starts[g], starts[g + 1]
            cnt = e1 - e0
            srcs = np.zeros(ETt, np.int64)
            srcs[:cnt] = src_pos[e0:e1]
            dl = np.full(ETt, 128, np.int64)
            dl[:cnt] = dst_s[e0:e1] - g * P
            o = int(offs[k])
            idx_arr[:, o * 8:(o + St) * 8] = _wrap_idx(srcs)
            dst_arr[:, o:o + St] = dl.reshape(St, P).T
            iv = np.zeros(ETt, np.float32)
            iv[:cnt] = invdeg_full[dst_s[e0:e1]]
            dst_arr[:, SC + o:SC + o + St] = iv.reshape(St, P).T
        # slot-permuted activations: slot k of core r holds local tile perms[r][k]
        pr = perms[r]

        xt_r = np.empty((P, DJ * sh), np.float32)
        for j in range(DJ):
            xs = xT[j * P:(j + 1) * P, r * sh:(r + 1) * sh]      # [P, sh]
            xs = xs.reshape(P, nt, P)[:, pr, :].reshape(P, sh)   # permute tiles
            xt_r[:, j * sh:(j + 1) * sh] = xs

        in_maps.append({
            "xt_in": xt_r.astype(ml_dtypes.bfloat16),
            "wpack_in": wpack,
            "vpack_in": vpack,
            "idx_in": idx_arr,
            "dst_in": dst_arr,
            "xtab_in": xtab,
        })
    return in_maps, perms, (n_pad, sh, nt, tuple(int(s) for s in S_list), scale, oma)


def kernel(**inputs):
    x = np.asarray(inputs["x"], np.float32)
    edge_index = np.asarray(inputs["edge_index"])
    args = dict(
        Wq=np.asarray(inputs["Wq"], np.float32), bq=np.asarray(inputs["bq"], np.float32),
        Wk=np.asarray(inputs["Wk"], np.float32), bk=np.asarray(inputs["bk"], np.float32),
        Wv=np.asarray(inputs["Wv"], np.float32), bv=np.asarray(inputs["bv"], np.float32),
        Ws=np.asarray(inputs["Ws"], np.float32), bs=np.asarray(inputs["bs"], np.float32),
        Wl=np.asarray(inputs["Wl"], np.float32), bl=np.asarray(inputs["bl"], np.float32),
        Wr=np.asarray(inputs["Wr"], np.float32),
        gamma=np.asarray(inputs["gamma"], np.float32),
        beta=np.asarray(inputs["beta"], np.float32),
        alpha_res=float(np.asarray(inputs["alpha_res"])),
    )
    src = edge_index[0].astype(np.int64)
    dst = edge_index[1].astype(np.int64)

    in_maps, perms, (n_pad, sh, nt, S_list, scale, oma) = _host_prep(x, src, dst, **args)
    t0 = time.time()
    nc = build_nc(n_pad, sh, nt, S_list, scale, oma)
    print(f"[kernel] build+compile {time.time()-t0:.1f}s", flush=True)
    t0 = time.time()
    res = run_bass_kernel_spmd(nc, in_maps, core_ids=list(range(NC)))
    print(f"[kernel] run {time.time()-t0:.1f}s", flush=True)
    # rows come back slot-ordered; un-permute to natural node order
    outs = []
    for r in range(NC):
        o = np.asarray(res.results[r]["out"]).astype(np.float32).reshape(nt, P, D)
        outs.append(o[np.argsort(perms[r])].reshape(sh, D))
    out = np.concatenate(outs, axis=0)
    return out[:x.shape[0]]


# revision 54
# speedup vs baseline: 1.0014x; 1.0014x over previous
"""Trainium2 Bass kernel for nn_MixGNN (TransformerConv + 3x SAGEConv + BN + gated residual).

Strategy (8 NeuronCores, dst-node sharding):
  - Pad N 10000 -> 10240; core r owns 1280 dst nodes = 10 tiles of 128. Each
    core's tile slots are sorted by edge count (descending) so the static SPMD
    per-slot chunk counts (max over cores) stay tight; all DRAM node tables
    are stored slot-ordered and gather indices are remapped to positions.
  - Host preprocessing (graph structure + parameter algebra only): sort edges
    by dst, bucket per dst-tile, pad to 128-edge chunks, wrapped int16 gather
    indices, per-chunk local-dst + invdeg-of-dst columns, packed bf16 weights.
    Attention is folded: M = Wq @ Wk.T / sqrt(d), so logits[e] =
    x[dst_e] @ M @ x[src_e]^T; the bk term is per-dst and cancels in softmax;
    bq is zero in this problem. BN gamma (eval mode) is folded into Wl/Wr
    columns; all biases enter PSUM via rank-1 ones-row matmuls.
  - Transformer pass: per tile, dma_gather the x-table twice (transposed for
    scores, rows for values); scores psc = xgT.T @ (M^T X_tile^T) on PE in
    4-chunk groups; exp on Act per group (no max-subtraction: logits O(1));
    w_b = (dst_e==n) * exp on DVE; value-agg + exp-sum denominator via
    indicator matmuls into PSUM; normalize, then post-multiply by Wv
    (linearity) together with the x @ Ws skip path.
  - SAGE pass: one gather per layer from the AllGathered bf16 h table;
    indicator build folds 1/deg (per-edge scalar); aggregation is TRANSPOSED
    (paggT[d,n] += vg[e,d]^T @ ind[e,n], separate PSUM banks per d-chunk) so
    the mean feeds Wl matmuls directly without PE transposes; h @ Wr uses a
    transposed h copy maintained per tile; gated residual + ReLU epilogue.
  - Gathers are split into ~4 even pieces per tile to pipeline SWDGE
    descriptor-gen (Pool) against DMA transfer.
  - Halo exchange: 3 AllGathers (h0, h1, h2) of 0.65MB/core bf16 shards.
Output: fp32 [10000, 256] (bf16 device output, upcast + slot-unpermuted on host).
"""
import os
import sys
import time

import numpy as np

for _p in ("/opt/trn_rl_repo",):
    if _p not in sys.path:
        sys.path.insert(0, _p)

import ml_dtypes  # noqa: E402
import concourse.bacc as bacc  # noqa: E402
import concourse.mybir as mybir  # noqa: E402
import concourse.tile as tile  # noqa: E402
from concourse.bass_utils import run_bass_kernel_spmd  # noqa: E402

P = 128
D = 256
DJ = D // P           # 2 d-chunks of 128
NC = 8                # cores
L = 3                 # SAGE layers
BN_EPS = 1e-5
N_AG = 3              # AllGathers on the critical path (h0, h1, h2)

F32 = mybir.dt.float32
BF16 = mybir.dt.bfloat16
I16 = mybir.dt.int16
V_DT = BF16           # gathered-table + indicator dtype
H_DT = BF16

_nc_cache = {}


def _wrap_idx(a):
    """[S*128] int array -> [128, S*8] int16 wrapped gather-index layout."""
    w16 = a.reshape(-1, 16).T.astype(np.int16)   # [16, S*8]
    return np.tile(w16, (8, 1))                  # replicate to 8 Q7 stripes


def build_nc(n_pad, sh, nt, S_list, scale, oma):
    stages = int(os.environ.get("KSTAGES", "5"))
    nocc = os.environ.get("KNOCC") == "1"
    ksm = int(os.environ.get("KSM", "12"))
    kgp = int(os.environ.get("KGP", "2"))
    kpsc = int(os.environ.get("KPSC", "3"))
    kptr = int(os.environ.get("KPTR", "2"))
    kpagg = int(os.environ.get("KPAGG", "2"))
    kpmm = int(os.environ.get("KPMM", "1"))
    khalf = int(os.environ.get("KHALF", "4"))  # gather splits per tile
    kabl = os.environ.get("KABL", "")
    S_list = tuple(int(s) for s in S_list)
    key = (n_pad, sh, nt, S_list, round(scale, 9), round(oma, 9), stages,
           nocc, ksm, kgp, kpsc, kptr, kpagg, kpmm, khalf, kabl,
           os.environ.get("KHALFT"),
           os.environ.get("KKGT"), os.environ.get("KVG"), os.environ.get("KPAIR"), os.environ.get("KGRAD"), os.environ.get("KTAIL"), os.environ.get("KTAIL2"))
    if key in _nc_cache:
        return _nc_cache[key]

    SC = sum(S_list)                 # total chunks across local tiles
    offs = [0]
    for s in S_list:
        offs.append(offs[-1] + s)
    ndev = 1 if nocc else NC
    nc = bacc.Bacc("TRN2", target_bir_lowering=False, debug=False, num_devices=ndev)

    NW = 9  # packed weights: M, Wv, Ws, Wl0, Wr0, Wl1, Wr1, Wl2, Wr2
    NV = 7  # packed vecs: bv+bs, Gx0, Bx0, Gx1, Bx1, Gx2, Bx2

    xt_in = nc.dram_tensor("xt_in", [P, DJ * sh], BF16, kind="ExternalInput")
    wpack_in = nc.dram_tensor("wpack_in", [P, NW * DJ * D], BF16, kind="ExternalInput")
    vpack_in = nc.dram_tensor("vpack_in", [1, NV * D], BF16, kind="ExternalInput")
    idx_in = nc.dram_tensor("idx_in", [P, SC * 8], I16, kind="ExternalInput")
    dst_in = nc.dram_tensor("dst_in", [P, 2 * SC], F32, kind="ExternalInput")
    xtab_in = nc.dram_tensor("xtab_in", [n_pad, D], BF16, kind="ExternalInput")
    out_dram = nc.dram_tensor("out", [sh, D], BF16, kind="ExternalOutput")

    WM, WV, WS = 0, 1, 2
    WL = [3, 5, 7]
    WR = [4, 6, 8]
    VBS = 0

    with tile.TileContext(nc) as tc:
        with (
            tc.tile_pool(name="cst", bufs=1) as cst,
            tc.tile_pool(name="sb", bufs=1) as sb,
            tc.tile_pool(name="g", bufs=kgp) as gp,
            tc.tile_pool(name="sm", bufs=ksm) as smp,
            tc.tile_pool(name="ps", bufs=2, space="PSUM") as ps,
            tc.tile_pool(name="dr", bufs=1, space="DRAM") as dr,
        ):
            # ---------------- constants / inputs to SBUF ----------------
            idx_sb = cst.tile([P, SC * 8], I16)
            _ic = S_list[0] * 8  # first tile's indices land first
            nc.sync.dma_start(out=idx_sb[:, :_ic], in_=idx_in[:, :_ic])
            nc.sync.dma_start(out=idx_sb[:, _ic:], in_=idx_in[:, _ic:])
            dstc = cst.tile([P, 2 * SC], F32)
            nc.sync.dma_start(out=dstc[:], in_=dst_in[:])
            wp = cst.tile([P, NW * DJ * D], BF16)
            nc.sync.dma_start(out=wp[:], in_=wpack_in[:])
            vp = cst.tile([1, NV * D], BF16)
            nc.sync.dma_start(out=vp[:], in_=vpack_in[:])
            xt = cst.tile([P, DJ * sh], BF16)
            for _xi in range(4):
                _c0 = _xi * (DJ * sh // 4)
                _c1 = (_xi + 1) * (DJ * sh // 4)
                nc.sync.dma_start(out=xt[:, _c0:_c1], in_=xt_in[:, _c0:_c1])

            iota_i = cst.tile([P, P], mybir.dt.int32)
            nc.gpsimd.iota(iota_i[:], pattern=[[1, P]], base=0, channel_multiplier=0)
            ones_v = cst.tile([P, 1], V_DT)
            nc.vector.memset(ones_v[:], 1.0)
            ones_row = cst.tile([1, P], BF16)
            nc.vector.memset(ones_row[:], 1.0)
            # identity for PE transposes: (iota_row == partition_idx)
            iota_part = cst.tile([P, 1], mybir.dt.int32)
            nc.gpsimd.iota(iota_part[:], pattern=[[1, 1]], base=0, channel_multiplier=1)
            iota_part_f = cst.tile([P, 1], F32)
            nc.vector.tensor_copy(out=iota_part_f[:], in_=iota_part[:])
            iota_f = cst.tile([P, P], F32)
            nc.vector.tensor_copy(out=iota_f[:], in_=iota_i[:])
            ident = cst.tile([P, P], F32)
            nc.vector.tensor_scalar(
                out=ident[:], in0=iota_f[:], scalar1=iota_part_f[:, :1], scalar2=None,
                op0=mybir.AluOpType.is_equal,
            )
            ident_b = cst.tile([P, P], BF16)
            nc.vector.tensor_copy(out=ident_b[:], in_=ident[:])
            iota_b = cst.tile([P, P], BF16)
            nc.vector.tensor_copy(out=iota_b[:], in_=iota_f[:])

            def wslice(w, j):
                return wp[:, (w * DJ + j) * D:(w * DJ + j + 1) * D]

            def vslice(k):
                return vp[:, k * D:(k + 1) * D]  # [1, D] single-partition row

            def xtile(j, t):
                return xt[:, j * sh + t * P: j * sh + (t + 1) * P]

            # ---------------- DRAM tables ----------------
            hag_in = [dr.tile([sh, D], H_DT, name=f"hag_in_{i}") for i in range(L)]
            h_full = [dr.tile([n_pad, D], H_DT, name=f"h_full_{i}",
                              addr_space=("Local" if nocc else "Shared"))
                      for i in range(L)]

            def allgather(in_t, out_t):
                if nocc:
                    pass  # per-tile h_full writes above stand in for the AG
                else:
                    nc.gpsimd.collective_compute(
                        "AllGather", mybir.AluOpType.bypass,
                        replica_groups=[list(range(NC))],
                        ins=[in_t[:]], outs=[out_t[:]],
                    )

            # ---------------- stage 0: aT = M^T X_tile^T per tile ----------------
            # aT[j][d, n] (j-th 128-row chunk of d) so that
            # psc[e, n] = sum_d xgT[d, e] * aT[d, n] = (x[src_e] @ M^T) . x[n]
            #           = x[n] @ M @ x[src_e]^T  (logit of edge e -> dst n)
            aT = [sb.tile([P, sh], BF16, name=f"aT_{j}") for j in range(DJ)]
            n0 = 0
            while n0 < sh:
                nn = min(512, sh - n0)
                for j in range(DJ):
                    pq = ps.tile([P, 512], F32, name="pq", tag="pmm", bufs=kpmm)
                    for ki in range(DJ):
                        nc.tensor.matmul(
                            pq[:, :nn],
                            lhsT=wslice(WM, ki)[:, j * P:(j + 1) * P],
                            rhs=xt[:, ki * sh + n0: ki * sh + n0 + nn],
                            start=(ki == 0), stop=(ki == DJ - 1),
                        )
                    nc.scalar.copy(out=aT[j][:, n0:n0 + nn], in_=pq[:, :nn])
                n0 += nn

            # shard-resident activations
            h_cur = sb.tile([P, nt * D], H_DT)
            h_nxt = sb.tile([P, nt * D], H_DT)
            hT_cur = sb.tile([P, DJ * sh], BF16)
            hT_nxt = sb.tile([P, DJ * sh], BF16)

            def agg_pass(layer, h_prev, hT_prev, h_out, hT_out):
                """layer -1: transformer (h_prev/hT_prev unused); 0..L-1: SAGE."""
                li = layer + 1  # h table index this pass WRITES (0 for transformer)
                kh = khalf if layer >= 0 else int(os.environ.get("KHALFT", "4"))
                for t in range(nt):
                    St = S_list[t]
                    ETt = St * P
                    o8 = offs[t] * 8
                    splits = []  # (c0, c1) chunk ranges per gather piece
                    c0 = 0
                    hi = St
                    tail_ws = []
                    _tw = os.environ.get("KTAIL", "4,2,2")
                    _tws = tuple(int(w) for w in _tw.split(",") if w)
                    if t == nt - 2 and os.environ.get("KTAIL2", "") and St > 6:
                        _t2 = tuple(int(w) for w in
                                    os.environ["KTAIL2"].split(",") if w)
                        tail_ws = list(_t2)
                        hi -= sum(_t2)
                    if t == nt - 1 and St > sum(_tws) + 2 and _tws:
                        tail_ws = list(_tws)
                        hi -= sum(_tws)
                    if t == 0:
                        _grad = tuple(int(w) for w in
                                      os.environ.get("KGRAD", "").split(",") if w)
                        for w in _grad:  # small leading pieces: lower latency
                            splits.append((c0, min(St, c0 + w)))
                            c0 += w
                            if c0 >= St:
                                break
                    base = max(1, (hi - c0 + kh - 1) // kh)
                    while c0 < hi:
                        ce = min(hi, c0 + base)
                        splits.append((c0, ce))
                        c0 = ce
                    for w in tail_ws:
                        splits.append((c0, c0 + w))
                        c0 += w
                    if layer < 0:
                        vg = gp.tile([P, St, D], V_DT, name="vg", tag="vg",
                                     bufs=int(os.environ.get("KVG", "3")))
                    else:
                        vg = gp.tile([P, St, D], H_DT, name="hg", tag="vg",
                                     bufs=int(os.environ.get("KVG", "3")))
                    kgt_pieces = []
                    if layer < 0:
                        ksplits = [s for s in splits]
                        nkg = 2 * kh + 4
                        for (ck, ce) in ksplits:
                            nn_k = (ce - ck) * P
                            nn_k = (ce - ck) * P
                            kgp_t = gp.tile([P, DJ, nn_k], BF16, name="kgt",
                                            tag="kgt", bufs=nkg)
                            nc.gpsimd.dma_gather(
                                out_ap=kgp_t[:],
                                in_ap=xtab_in[:],
                                idxs_ap=idx_sb[:, o8 + ck * 8: o8 + ce * 8],
                                num_idxs=nn_k, num_idxs_reg=nn_k, elem_size=D,
                                transpose=True, single_packet=False)
                            kgt_pieces.append((ck, ce, kgp_t))
                    src_tab = xtab_in if layer < 0 else h_full[layer]
                    for (ca, cb) in splits:
                        nn_i = (cb - ca) * P
                        idx_t = idx_sb[:, o8 + ca * 8: o8 + cb * 8]
                        nc.gpsimd.dma_gather(
                            out_ap=vg[:, ca:cb, :], in_ap=src_tab[:], idxs_ap=idx_t,
                            num_idxs=nn_i, num_idxs_reg=nn_i, elem_size=D,
                            single_packet=False)

                    if layer < 0:
                        pagg = ps.tile([P, D + 1], F32, name="pagg", tag="pagg",
                                       bufs=kpagg)
                        pz = ps.tile([P, D], F32, name="pz", tag="pmm", bufs=kpmm)
                        nc.tensor.matmul(pz[:], lhsT=ones_row[:], rhs=vslice(VBS),
                                         start=True, stop=False)
                        for j in range(DJ):
                            nc.tensor.matmul(pz[:], lhsT=xtile(j, t),
                                             rhs=wslice(WS, j),
                                             start=False, stop=False)
                    else:
                        # transposed agg: separate PSUM tiles per d-chunk
                        # (start=True zeroes a whole bank; slices can't share)
                        paggT = [ps.tile([P, P], F32, name=f"paggT{j}", tag="psc",
                                         bufs=kpsc) for j in range(DJ)]
                        pz = ps.tile([P, D], F32, name="pz", tag="pmm", bufs=kpmm)
                        nc.tensor.matmul(pz[:], lhsT=ones_row[:],
                                         rhs=vslice(2 + 2 * layer),
                                         start=True, stop=False)
                        for j in range(DJ):
                            nc.tensor.matmul(
                                pz[:],
                                lhsT=hT_prev[:, j * sh + t * P: j * sh + (t + 1) * P],
                                rhs=wslice(WR[layer], j),
                                start=False, stop=False)
                    if layer < 0:
                        # chunk pairs: one [P,2P] exp per two chunks (halves
                        # the Act per-instruction init overhead)
                        kpair = int(os.environ.get("KPAIR", "4"))
                        cp = 0
                        while cp < St:
                            npair = min(kpair, St - cp)
                            psc = ps.tile([P, npair * P], F32, name="psc",
                                          tag="psc", bufs=kpsc)
                            for ci in range(npair):
                                c = cp + ci
                                kge = next(p for p in kgt_pieces
                                           if p[0] <= c < p[1])
                                cof = c - kge[0]
                                for j in range(DJ):
                                    nc.tensor.matmul(
                                        psc[:, ci * P:(ci + 1) * P],
                                        lhsT=kge[2][:, j, cof * P:(cof + 1) * P],
                                        rhs=aT[j][:, t * P:(t + 1) * P],
                                        start=(j == 0), stop=(j == DJ - 1))
                            exps = smp.tile([P, npair * P], BF16, name="exps")
                            nc.scalar.activation(exps[:], psc[:],
                                                 mybir.ActivationFunctionType.Exp)
                            for ci in range(npair):
                                c = cp + ci
                                dcol = dstc[:, offs[t] + c: offs[t] + c + 1]
                                w_b = smp.tile([P, P], V_DT, name="w_b", tag="w_b")
                                nc.vector.scalar_tensor_tensor(
                                    out=w_b[:], in0=iota_b[:], scalar=dcol,
                                    in1=exps[:, ci * P:(ci + 1) * P],
                                    op0=mybir.AluOpType.is_equal,
                                    op1=mybir.AluOpType.mult)
                                nc.tensor.matmul(pagg[:, :D], lhsT=w_b[:],
                                                 rhs=vg[:, c, :],
                                                 start=(c == 0), stop=(c == St - 1))
                                nc.tensor.matmul(pagg[:, D:D + 1], lhsT=w_b[:],
                                                 rhs=ones_v[:],
                                                 start=False, stop=(c == St - 1))
                            cp += npair
                    else:
                        for c in range(St):
                            dcol = dstc[:, offs[t] + c: offs[t] + c + 1]
                            ivcol = dstc[:, SC + offs[t] + c: SC + offs[t] + c + 1]
                            ind_b = smp.tile([P, P], H_DT, name="ind_b", tag="w_b")
                            nc.vector.tensor_scalar(
                                out=ind_b[:], in0=iota_b[:], scalar1=dcol,
                                scalar2=ivcol, op0=mybir.AluOpType.is_equal,
                                op1=mybir.AluOpType.mult)
                            for j in range(DJ):
                                nc.tensor.matmul(
                                    paggT[j][:],
                                    lhsT=vg[:, c, j * P:(j + 1) * P],
                                    rhs=ind_b[:],
                                    start=(c == 0), stop=(c == St - 1))

                    # ---- tile epilogue -> h_out tile [node, d] ----
                    if layer < 0:
                        smax = smp.tile([P, 1], F32, name="smax")
                        nc.vector.tensor_scalar(
                            out=smax[:], in0=pagg[:, D:D + 1], scalar1=1e-30,
                            scalar2=None, op0=mybir.AluOpType.max)
                        rs = smp.tile([P, 1], F32, name="rs")
                        nc.vector.reciprocal(rs[:], smax[:])
                        # mean_x = (sum_e attn * x[src]) / denom, then
                        # h = relu(mean_x @ Wv + x @ Ws + (bv + bs))
                        mean_x = smp.tile([P, D], BF16, name="mean_x", tag="t1")
                        nc.scalar.activation(mean_x[:], pagg[:, :D],
                                             mybir.ActivationFunctionType.Copy,
                                             scale=rs[:, :1])
                        for j in range(DJ):
                            ptr = ps.tile([P, P], BF16, name="ptr", tag="ptr", bufs=kptr)
                            nc.tensor.transpose(out=ptr[:],
                                                in_=mean_x[:, j * P:(j + 1) * P],
                                                identity=ident_b[:])
                            mT = smp.tile([P, P], BF16, name="mT", tag="mT")
                            nc.scalar.copy(out=mT[:], in_=ptr[:])
                            nc.tensor.matmul(pz[:], lhsT=mT[:],
                                             rhs=wslice(WV, j),
                                             start=False, stop=(j == DJ - 1))
                        nc.scalar.activation(h_out[:, t * D:(t + 1) * D], pz[:],
                                             mybir.ActivationFunctionType.Relu)
                        hfin = None
                    else:
                        for j in range(DJ):
                            mT = smp.tile([P, P], BF16, name="mT", tag="mT")
                            nc.scalar.copy(out=mT[:], in_=paggT[j][:])
                            nc.tensor.matmul(pz[:], lhsT=mT[:],
                                             rhs=wslice(WL[layer], j),
                                             start=False, stop=(j == DJ - 1))
                        t3 = smp.tile([P, D], F32, name="t3s", tag="t4")
                        nc.vector.scalar_tensor_tensor(
                            out=t3[:], in0=h_prev[:, t * D:(t + 1) * D], scalar=oma,
                            in1=pz[:], op0=mybir.AluOpType.mult,
                            op1=mybir.AluOpType.add)
                        if layer < L - 1:
                            nc.scalar.activation(h_out[:, t * D:(t + 1) * D], t3[:],
                                                 mybir.ActivationFunctionType.Relu)
                        else:
                            hfin = smp.tile([P, D], BF16, name="hfin", tag="t1")
                            nc.scalar.activation(hfin[:], t3[:],
                                                 mybir.ActivationFunctionType.Relu)

                    if layer < L - 1:
                        if nocc:
                            # sim stand-in: the collective's local table write,
                            # fed straight from the shard epilogue
                            nc.sync.dma_start(out=h_full[li][t * P:(t + 1) * P, :],
                                              in_=h_out[:, t * D:(t + 1) * D])
                        else:
                            nc.sync.dma_start(out=hag_in[li][t * P:(t + 1) * P, :],
                                              in_=h_out[:, t * D:(t + 1) * D])
                        for j in range(DJ):
                            ptr2 = ps.tile([P, P], H_DT, name="ptr2", tag="ptr", bufs=kptr)
                            nc.tensor.transpose(
                                out=ptr2[:],
                                in_=h_out[:, t * D + j * P: t * D + (j + 1) * P],
                                identity=ident_b[:])
                            nc.scalar.copy(
                                out=hT_out[:, j * sh + t * P: j * sh + (t + 1) * P],
                                in_=ptr2[:])
                    else:
                        nc.sync.dma_start(out=out_dram[t * P:(t + 1) * P, :],
                                          in_=hfin[:])

                if layer < L - 1:
                    allgather(hag_in[li], h_full[li])

            if stages <= 1:
                # dump a slice so the program has an output
                tmpo = smp.tile([P, D], F32, name="tmpo")
                for t in range(nt):
                    nc.vector.tensor_copy(out=tmpo[:], in_=xt[:, :D])
                    nc.sync.dma_start(out=out_dram[t * P:(t + 1) * P, :], in_=tmpo[:])
            else:
                agg_pass(-1, None, None, h_cur, hT_cur)
                bufs = [(h_cur, hT_cur), (h_nxt, hT_nxt)]
                for i in range(min(L, stages - 2)):
                    h_prev, hT_prev = bufs[i % 2]
                    h_out, hT_out = bufs[(i + 1) % 2]
                    agg_pass(i, h_prev, hT_prev, h_out, hT_out)
                if stages - 2 < L:
                    hsrc, _ = bufs[max(0, stages - 2) % 2]
                    for t in range(nt):
                        nc.sync.dma_start(out=out_dram[t * P:(t + 1) * P, :],
                                          in_=hsrc[:, t * D:(t + 1) * D])

    nc.compile()
    _nc_cache[key] = nc
    return nc


def _host_prep(x, src, dst, Wq, bq, Wk, bk, Wv, bv, Ws, bs, Wl, bl, Wr,
               gamma, beta, alpha_res):
    n, d = x.shape
    n_pad = ((n + NC * P - 1) // (NC * P)) * (NC * P)
    sh = n_pad // NC
    nt = sh // P
    n_tiles = n_pad // P

    order = np.argsort(dst, kind="stable")
    src_s, dst_s = src[order], dst[order]
    tile_of = dst_s // P
    counts = np.bincount(tile_of, minlength=n_tiles)
    starts = np.concatenate([[0], np.cumsum(counts)])

    # Per-core slot assignment: sort each core's local tiles by edge count
    # (descending) so slot k holds every core's k-th busiest tile. The static
    # SPMD chunk count per slot is then the max over cores, which is tight.
    perms = []   # perms[r][k] = local tile index of core r in slot k
    s_sorted = np.empty((NC, nt), np.int64)
    for r in range(NC):
        c_r = counts[r * nt:(r + 1) * nt]
        p_r = np.argsort(-c_r, kind="stable")
        perms.append(p_r)
        s_sorted[r] = (c_r[p_r] + P - 1) // P
    S_list = np.maximum(s_sorted.max(axis=0), 1).astype(np.int64)
    SC = int(S_list.sum())
    offs = np.concatenate([[0], np.cumsum(S_list)]).astype(np.int64)

    # All DRAM node tables (xtab, h_full via hag_in writes) are slot-ordered:
    # position (r*nt + k)*P + p holds node (r*nt + perms[r][k])*P + p. Gather
    # indices address table positions, so remap node ids -> positions.
    invperms = [np.argsort(p) for p in perms]
    pos_of_tile = np.empty(n_tiles, np.int64)
    for r in range(NC):
        pos_of_tile[r * nt:(r + 1) * nt] = r * nt + invperms[r]
    ar = np.arange(n_pad)
    pos_of_node = pos_of_tile[ar // P] * P + (ar % P)
    src_pos = pos_of_node[src_s]

    deg = np.bincount(dst, minlength=n_pad).astype(np.float32)
    invdeg_full = 1.0 / np.maximum(deg, 1.0)

    al = 1.0 / (1.0 + np.exp(-alpha_res))
    oma = float(1.0 - al)
    bn_scale = 1.0 / np.sqrt(1.0 + BN_EPS)
    scale = 1.0 / np.sqrt(float(d))

    x_pad = np.zeros((n_pad, D), np.float32)
    x_pad[:n] = x
    xT = x_pad.T.copy()
    xtab = np.zeros((n_pad, D), ml_dtypes.bfloat16)
    xtab[pos_of_node] = x_pad.astype(ml_dtypes.bfloat16)

    # fold attention: logits = scale * (x[dst] @ Wq + bq) . (x[src] @ Wk + bk)
    #   = x[dst] @ M @ x[src]^T  (+ per-dst const, cancels in softmax; bq = 0)
    M = (Wq @ Wk.T) * scale
    Gx = [al * bn_scale * gamma[i] for i in range(L)]
    Wlg = [Wl[i] * Gx[i][None, :] for i in range(L)]
    Wrg = [Wr[i] * Gx[i][None, :] for i in range(L)]
    weights = [M, Wv, Ws, Wlg[0], Wrg[0], Wlg[1], Wrg[1], Wlg[2], Wrg[2]]
    NW = len(weights)
    wpack = np.empty((P, NW * DJ * D), np.float32)
    for w, W in enumerate(weights):
        for j in range(DJ):
            wpack[:, (w * DJ + j) * D:(w * DJ + j + 1) * D] = W[j * P:(j + 1) * P, :]
    wpack = wpack.astype(ml_dtypes.bfloat16)

    Bx = [al * (bl[i] * bn_scale * gamma[i] + beta[i]) for i in range(L)]
    vecs = [bv + bs, Bx[0], Bx[0], Bx[1], Bx[1], Bx[2], Bx[2]]
    vpack = np.concatenate(vecs)[None, :].astype(ml_dtypes.bfloat16)

    in_maps = []
    for r in range(NC):
        idx_arr = np.zeros((P, SC * 8), np.int16)
        dst_arr = np.full((P, 2 * SC), 128.0, np.float32)
        dst_arr[:, SC:] = 0.0
        for k in range(nt):
            tloc = int(perms[r][k])
            St = int(S_list[k])
            ETt = St * P
            g = r * nt + tloc
            e0, e1 = starts[g], starts[g + 1]
            cnt = e1 - e0
            srcs = np.zeros(ETt, np.int64)
            srcs[:cnt] = src_pos[e0:e1]
            dl = np.full(ETt, 128, np.int64)
            dl[:cnt] = dst_s[e0:e1] - g * P
            o = int(offs[k])
            idx_arr[:, o * 8:(o + St) * 8] = _wrap_idx(srcs)
            dst_arr[:, o:o + St] = dl.reshape(St, P).T
            iv = np.zeros(ETt, np.float32)
            iv[:cnt] = invdeg_full[dst_s[e0:e1]]
            dst_arr[:, SC + o:SC + o + St] = iv.reshape(St, P).T
        # slot-permuted activations: slot k of core r holds local tile perms[r][k]
        pr = perms[r]

        xt_r = np.empty((P, DJ * sh), np.float32)
        for j in range(DJ):
            xs = xT[j * P:(j + 1) * P, r * sh:(r + 1) * sh]      # [P, sh]
            xs = xs.reshape(P, nt, P)[:, pr, :].reshape(P, sh)   # permute tiles
            xt_r[:, j * sh:(j + 1) * sh] = xs

        in_maps.append({
            "xt_in": xt_r.astype(ml_dtypes.bfloat16),
            "wpack_in": wpack,
            "vpack_in": vpack,
            "idx_in": idx_arr,
            "dst_in": dst_arr,
            "xtab_in": xtab,
        })
    return in_maps, perms, (n_pad, sh, nt, tuple(int(s) for s in S_list), scale, oma)


def kernel(**inputs):
    x = np.asarray(inputs["x"], np.float32)
    edge_index = np.asarray(inputs["edge_index"])
    args = dict(
        Wq=np.asarray(inputs["Wq"], np.float32), bq=np.asarray(inputs["bq"], np.float32),
        Wk=np.asarray(inputs["Wk"], np.float32), bk=np.asarray(inputs["bk"], np.float32),
        Wv=np.asarray(inputs["Wv"], np.float32), bv=np.asarray(inputs["bv"], np.float32),
        Ws=np.asarray(inputs["Ws"], np.float32), bs=np.asarray(inputs["bs"], np.float32),
        Wl=np.asarray(inputs["Wl"], np.float32), bl=np.asarray(inputs["bl"], np.float32),
        Wr=np.asarray(inputs["Wr"], np.float32),
        gamma=np.asarray(inputs["gamma"], np.float32),
        beta=np.asarray(inputs["beta"], np.float32),
        alpha_res=float(np.asarray(inputs["alpha_res"])),
    )
    src = edge_index[0].astype(np.int64)
    dst = edge_index[1].astype(np.int64)

    in_maps, perms, (n_pad, sh, nt, S_list, scale, oma) = _host_prep(x, src, dst, **args)
    t0 = time.time()
    nc = build_nc(n_pad, sh, nt, S_list, scale, oma)
    print(f"[kernel] build+compile {time.time()-t0:.1f}s", flush=True)
    t0 = time.time()
    res = run_bass_kernel_spmd(nc, in_maps, core_ids=list(range(NC)))
    print(f"[kernel] run {time.time()-t0:.1f}s", flush=True)
    # rows come back slot-ordered; un-permute to natural node order
    outs = []
    for r in range(NC):
        o = np.asarray(res.results[r]["out"]).astype(np.float32).reshape(nt, P, D)
        outs.append(o[np.argsort(perms[r])].reshape(sh, D))
    out = np.concatenate(outs, axis=0)
    return out[:x.shape[0]]
